# revision 52
# baseline (speedup 1.0000x reference)
"""DenseQConv1D Trainium2 kernel.

Math: the reference computes, per output channel c and patch p (128-dim im2col
column of x, normalized):
    out[c,p] = sum_e sign(e) * (s_p^T (E @ R_c)[:128,:])_e^2
with R_c = kron of 9 RY(theta[c,q]) rotations and sign(e) = Z on the MSB qubit.
Because every RY factor is orthogonal and the measurement only touches qubit 0,
with E128 = E[:128,:], F = E128[:,:256], G = E128[:,256:]:
    GZ = F F^T - G G^T,  GX = F G^T + G F^T   (both 128x128, theta-independent)
    out[c,p] = (cos t_c * p^T GZ p + sin t_c * p^T GX p) / ||p||^2,  t = theta[c,0]

For the ring-of-CNOTs entangle matrix E is a PERMUTATION matrix, under which
GZ collapses to diag(s) with s in {+-1} and GX == 0 identically (verified
against the jax reference to ~9e-7).  So, with patch dim d = c*8 + j,
    out[c~, l] = cos(theta[c~,0]) * zrow[l] / n2[l]
    zrow[l] = sum_j sum_c s[c*8+j] x[c, l+j]^2 ,  n2[l] = sum_j sum_c x[c,l+j]^2
The host computes s and cos(theta[:,0]) from the actual inputs (cheap numpy)
and falls back to an exact dense numpy evaluation if the structure ever fails
to hold.

Device kernel (per core, batch b): x is loaded ONCE (no 8x im2col DMA
duplication) in an 8-block layout xblk[(m,c), l'] = x[c, 128m + l'] with a
7-column halo; squares go to bf16 on DVE; the j-shifts of the im2col become
8 PSUM-accumulating bf16 matmuls with a block-diagonal stationary that
computes zrow (PSUM rows 0-7) and n2 (rows 32-39 - engines may only read
PSUM/SBUF at 32-aligned partition starts) in one pass:
    zn40 += wallC[:, 40j:40j+40]^T @ x2[:, j:j+128]
then a custom-DVE fast reciprocal, ratio = zrow*inv (bf16, block layout),
and a single K=8 matmul against a host-built cos*delta matrix that both
broadcasts cos over the 16 output channels and re-labels the blocks:
out_blk[c*8+m, l'] = cos_c * ratio[m, l']; the out-DMA's DRAM access
pattern un-blocks the layout.  The kernel has no ACT ops (no PWP table
loads); a dummy-matmul chain at kernel start warms the PE clock (HAM)
while the DMAs stream.  All host preprocessing (padding, the +-1/cos
stationaries) is O(KB) numpy glue; every per-element x computation runs
on device.

Sharding: batch dimension across the 8 cores (core b computes x[b]).
"""

import numpy as np

B = 8
C_IN = 16
C_OUT = 16
L = 1024
K = 8
L_OUT = L - K + 1  # 1017
LP = 1024  # padded patch count per core (cols 1017:1024 are dummy)
P = 128  # patch vector length = C_IN*K = partitions
LXP = 1040  # host-padded x row length (cols 1024: = 1.0)
NBLK = 8  # l-blocks of 128
HALO = 135  # 128 + K - 1
N_WARM = 22  # PE warmup matmuls

_CACHE = {}


def _build_nc():
    import bass_rust as _br
    import concourse.bacc as bacc
    import concourse.mybir as mybir
    import concourse.tile as tile

    f32 = mybir.dt.float32
    bf16 = mybir.dt.bfloat16

    nc = bacc.Bacc("TRN2", target_bir_lowering=False, debug=False)

    # single packed input: per partition p=(m,c), cols 0-134 are the x block
    # row xp[c, 128m:128m+135] (f32) and cols 135-358 hold that partition's
    # 448 wallC bf16 entries bit-packed into 224 f32 words
    xw_ext = nc.declare_dram_parameter("xw", [P, 359], f32, isOutput=False)
    out_ext = nc.declare_dram_parameter("out", [C_OUT, LP], bf16, isOutput=True)

    with tile.TileContext(nc) as tc, tc.tile_pool(name="sb", bufs=1) as sb, \
            tc.tile_pool(name="psW", bufs=1, space="PSUM") as psW, \
            tc.tile_pool(name="psZ", bufs=1, space="PSUM") as psZ, \
            tc.tile_pool(name="psO", bufs=2, space="PSUM") as psO:
        # ---- PE warmup: release the HAM clock throttle while DMAs stream.
        warm = sb.tile([P, P], bf16)
        nc.vector.memset(warm[:], 0.0)
        wps = psW.tile([1, P], f32, tag="w")
        for _ in range(N_WARM):
            nc.tensor.matmul(
                wps[:], warm[:, 0:1], warm[:], start=True, stop=True
            )

        # ---- ONE packed input DMA per ring: x block rows + the per-shift
        # stationaries + the cos-combine matrix arrive under a single
        # completion semaphore per ring (per-call completion costs ~2.5us,
        # so fewer calls on the gate wins).
        # wall cols 40j..40j+39 (bf16 view): ones at 0-7 so n2 lands at PSUM
        # partition 0 for the custom-DVE reciprocal, zero gap, z_j at 32-39
        # read by a regular tensor_mul (which honors partition offsets);
        # bf16 cols 320-447: the cos-combine matrix on partitions 0-7.
        xw = sb.tile([P, 359], f32)
        nc.sync.dma_start(xw[0:64, :], xw_ext[0:64, :])
        nc.scalar.dma_start(xw[64:P, :], xw_ext[64:P, :])

        def wallj(j):
            return xw[:, 135 + 20 * j : 155 + 20 * j].bitcast(bf16)

        # ---- squares (bf16, split so each half starts on its ring's DMA)
        x2b = sb.tile([P, HALO], bf16)
        nc.vector.tensor_mul(x2b[0:64, :], xw[0:64, 0:HALO], xw[0:64, 0:HALO])
        nc.vector.tensor_mul(x2b[64:P, :], xw[64:P, 0:HALO], xw[64:P, 0:HALO])

        # n2 (rows 0-7) and zrow (rows 32-39) accumulate in one PSUM tile
        zn40 = psZ.tile([40, P], f32, tag="zn")
        for j in range(K):
            nc.tensor.matmul(
                zn40[:], wallj(j), x2b[:, j : j + P],
                start=(j == 0), stop=(j == K - 1),
            )

        # ---- ratio in block layout (no ACT, so no PWP table loads at all).
        # The custom-DVE reciprocal drops partition offsets on its input, so
        # n2 sits at rows 0-7; the regular tensor_mul handles the offset-32
        # zrow read correctly.
        inv = sb.tile([NBLK, P], f32)
        nc.vector.reciprocal_approx_fast(inv[:], zn40[0:NBLK, :])
        ratio = sb.tile([NBLK, P], bf16)
        nc.vector.tensor_mul(ratio[:], zn40[32:40, :], inv[:])

        # ---- out_blk[c*8+m, l'] = cos_c * ratio[m, l'] in ONE K=8 matmul;
        # the out-DMA's DRAM access pattern un-blocks the layout.
        outp = psO.tile([P, P], f32, tag="o")
        nc.tensor.matmul(
            outp[:], xw[0:NBLK, 295:359].bitcast(bf16), ratio[:],
            start=True, stop=True,
        )
        # evac + out-DMA split by halves so the first DMA issues earlier
        # and the two transfers pipeline
        outs = sb.tile([P, P], bf16)
        oap0 = _br.AP(out_ext, 0, [[LP, 8], [P, NBLK], [1, P]])
        oap1 = _br.AP(out_ext, 8 * LP, [[LP, 8], [P, NBLK], [1, P]])
        nc.vector.tensor_copy(outs[0:64, :], outp[0:64, :])
        nc.sync.dma_start(oap0, outs[0:64, :])
        nc.vector.tensor_copy(outs[64:P, :], outp[64:P, :])
        nc.scalar.dma_start(oap1, outs[64:P, :])

    nc.compile()
    return nc


def _structure(ent):
    """Return (s, ok): s = diag sign vector (patch-dim order) if the
    entangle matrix has the permutation structure that kills GX."""
    E128 = ent[:P, :]
    F, G = E128[:, :256], E128[:, 256:]
    GZ = F @ F.T - G @ G.T
    GX = F @ G.T + G @ F.T
    s = np.diag(GZ).copy()
    ok = (
        np.abs(GX).max() <= 1e-5
        and np.abs(GZ - np.diag(s)).max() <= 1e-5
        and np.abs(np.abs(s) - 1.0).max() <= 1e-5
    )
    return s, ok


def _fallback(x, theta, ent):
    """Exact dense numpy evaluation (never taken for the reference E)."""
    DIM = 512
    NQ = 9

    def ry(t):
        c, s = np.cos(t / 2.0), np.sin(t / 2.0)
        return np.array([[c, -s], [s, c]], dtype=np.float64)

    Rs = []
    for c in range(C_OUT):
        R = ry(theta[c, 0])
        for q in range(1, NQ):
            R = np.kron(R, ry(theta[c, q]))
        Rs.append(R)
    Rs = np.stack(Rs)
    Us = np.einsum("ij,cjk->cik", ent.astype(np.float64), Rs)
    idx = np.arange(L_OUT)[:, None] + np.arange(K)[None, :]
    patches = x[:, :, idx].transpose(0, 2, 1, 3).reshape(-1, C_IN * K)
    norms = np.maximum(np.linalg.norm(patches, axis=-1, keepdims=True), 1e-12)
    states = np.zeros((patches.shape[0], DIM))
    states[:, : C_IN * K] = patches / norms
    psi = np.einsum("cde,nd->cne", Us, states)
    bit0 = (np.arange(DIM) >> (NQ - 1)) & 1
    sign = np.where(bit0 == 0, 1.0, -1.0)
    out = np.einsum("cne,e->cn", psi * psi, sign)
    return (
        out.reshape(C_OUT, B, L_OUT).transpose(1, 0, 2).astype(np.float32)
    )


def _host_inputs(x, theta, s):
    import ml_dtypes

    xp = np.ones((B, C_IN, LXP), dtype=np.float32)
    xp[:, :, :L] = x
    # bf16 cols 40j+m' : ones (m==m'); cols 40j+32+m' : z_j (s[c*8+j]*(m==m'))
    # bf16 cols 320 + c*8+m~ on partition row m: cos(theta[c,0]) * (m==m~)
    wallC = np.zeros((P, 448), dtype=np.float32)
    m = np.arange(P) // 16
    c = np.arange(P) % 16
    for j in range(K):
        wallC[np.arange(P), 40 * j + m] = 1.0
        wallC[np.arange(P), 40 * j + 32 + m] = s[c * 8 + j]
    cos = np.cos(theta[:, 0])
    pc = np.arange(P) // NBLK
    pm = np.arange(P) % NBLK
    wallC[pm, 320 + np.arange(P)] = cos[pc]
    wall_f32 = (
        np.ascontiguousarray(wallC.astype(ml_dtypes.bfloat16))
        .view(np.uint16)
        .view(np.float32)
    )  # [P, 224] bf16 bit-pack
    # pack per-core: [P, 359] = x block rows (0:135) + wall words (135:359)
    xw = np.empty((B, P, 359), dtype=np.float32)
    for mm in range(NBLK):
        for cc in range(C_IN):
            p = mm * 16 + cc
            xw[:, p, 0:HALO] = xp[:, cc, 128 * mm : 128 * mm + HALO]
    xw[:, :, HALO:359] = wall_f32[None, :, :]
    return xw


def _run(inputs, trace=False):
    from concourse.bass_utils import run_bass_kernel_spmd

    x = np.ascontiguousarray(np.asarray(inputs["x"], dtype=np.float32))
    theta = np.asarray(inputs["theta"], dtype=np.float32)
    ent = np.asarray(inputs["entangle_matrix"], dtype=np.float32)

    s, ok = _structure(ent)
    if not ok:  # pragma: no cover - safety net for non-reference entanglers
        return _fallback(x, theta, ent), None

    xw = _host_inputs(x, theta, s)

    if "nc" not in _CACHE:
        _CACHE["nc"] = _build_nc()
    nc = _CACHE["nc"]

    in_maps = [{"xw": np.ascontiguousarray(xw[b])} for b in range(B)]
    res = run_bass_kernel_spmd(
        nc, in_maps, core_ids=list(range(B)), trace=trace
    )
    out = np.stack([res.results[b]["out"][:, :L_OUT] for b in range(B)], axis=0)
    return np.ascontiguousarray(out.astype(np.float32)), res


def kernel(**inputs):
    out, _ = _run(inputs)
    return out


# revision 57
# speedup vs baseline: 1.0160x; 1.0160x over previous
"""DenseQConv1D Trainium2 kernel.

Math: the reference computes, per output channel c and patch p (128-dim im2col
column of x, normalized):
    out[c,p] = sum_e sign(e) * (s_p^T (E @ R_c)[:128,:])_e^2
with R_c = kron of 9 RY(theta[c,q]) rotations and sign(e) = Z on the MSB qubit.
Because every RY factor is orthogonal and the measurement only touches qubit 0,
with E128 = E[:128,:], F = E128[:,:256], G = E128[:,256:]:
    GZ = F F^T - G G^T,  GX = F G^T + G F^T   (both 128x128, theta-independent)
    out[c,p] = (cos t_c * p^T GZ p + sin t_c * p^T GX p) / ||p||^2,  t = theta[c,0]

For the ring-of-CNOTs entangle matrix E is a PERMUTATION matrix, under which
GZ collapses to diag(s) with s in {+-1} and GX == 0 identically (verified
against the jax reference to ~9e-7).  So, with patch dim d = c*8 + j,
    out[c~, l] = cos(theta[c~,0]) * zrow[l] / n2[l]
    zrow[l] = sum_j sum_c s[c*8+j] x[c, l+j]^2 ,  n2[l] = sum_j sum_c x[c,l+j]^2
The host computes s and cos(theta[:,0]) from the actual inputs (cheap numpy)
and falls back to an exact dense numpy evaluation if the structure ever fails
to hold.

Device kernel (per core, batch b): x is loaded ONCE (no 8x im2col DMA
duplication) in an 8-block layout xblk[(m,c), l'] = x[c, 128m + l'] with a
7-column halo; squares go to bf16 on DVE; the j-shifts of the im2col become
8 PSUM-accumulating bf16 matmuls with a block-diagonal stationary that
computes zrow (PSUM rows 0-7) and n2 (rows 32-39 - engines may only read
PSUM/SBUF at 32-aligned partition starts) in one pass:
    zn40 += wallC[:, 40j:40j+40]^T @ x2[:, j:j+128]
then a custom-DVE fast reciprocal, ratio = zrow*inv (bf16, block layout),
and a single K=8 matmul against a host-built cos*delta matrix that both
broadcasts cos over the 16 output channels and re-labels the blocks:
out_blk[c*8+m, l'] = cos_c * ratio[m, l']; the out-DMA's DRAM access
pattern un-blocks the layout.  The kernel has no ACT ops (no PWP table
loads); a dummy-matmul chain at kernel start warms the PE clock (HAM)
while the DMAs stream.  All host preprocessing (padding, the +-1/cos
stationaries) is O(KB) numpy glue; every per-element x computation runs
on device.

Sharding: batch dimension across the 8 cores (core b computes x[b]).
"""

import numpy as np

B = 8
C_IN = 16
C_OUT = 16
L = 1024
K = 8
L_OUT = L - K + 1  # 1017
LP = 1024  # padded patch count per core (cols 1017:1024 are dummy)
P = 128  # patch vector length = C_IN*K = partitions
LXP = 1040  # host-padded x row length (cols 1024: = 1.0)
NBLK = 8  # l-blocks of 128
HALO = 135  # 128 + K - 1
N_WARM = 22  # PE warmup matmuls

_CACHE = {}


def _build_nc():
    import bass_rust as _br
    import concourse.bacc as bacc
    import concourse.mybir as mybir
    import concourse.tile as tile

    f32 = mybir.dt.float32
    bf16 = mybir.dt.bfloat16

    nc = bacc.Bacc("TRN2", target_bir_lowering=False, debug=False)

    x_ext = nc.declare_dram_parameter("xp", [C_IN, LXP], f32, isOutput=False)
    w_ext = nc.declare_dram_parameter("wallC", [P, 448], bf16, isOutput=False)
    out_ext = nc.declare_dram_parameter("out", [C_OUT, LP], bf16, isOutput=True)

    with tile.TileContext(nc) as tc, tc.tile_pool(name="sb", bufs=1) as sb, \
            tc.tile_pool(name="psW", bufs=1, space="PSUM") as psW, \
            tc.tile_pool(name="psZ", bufs=1, space="PSUM") as psZ, \
            tc.tile_pool(name="psO", bufs=2, space="PSUM") as psO:
        # ---- PE warmup: release the HAM clock throttle while DMAs stream.
        warm = sb.tile([P, P], bf16)
        nc.vector.memset(warm[:], 0.0)
        wps = psW.tile([1, P], f32, tag="w")
        for _ in range(N_WARM):
            nc.tensor.matmul(
                wps[:], warm[:, 0:1], warm[:], start=True, stop=True
            )

        # ---- inputs: x once (8-block layout with halo) split across the two
        # HWDGE rings; wallC pipelined behind on ring A.
        # xblk[(m,c), l'] = xp[c, 128m + l'];  descriptors are 540B/partition
        xblk = sb.tile([P, HALO], f32)
        xap0 = _br.AP(x_ext, 0, [[128, 4], [LXP, C_IN], [1, HALO]])
        xap1 = _br.AP(x_ext, 128 * 4, [[128, 4], [LXP, C_IN], [1, HALO]])
        nc.sync.dma_start(xblk[0:64, :], xap0)
        nc.scalar.dma_start(xblk[64:P, :], xap1)

        # wallC cols 40j..40j+39: per-shift stationary (ones at 0-7 so n2
        # lands at PSUM partition 0 for the custom-DVE reciprocal, zero gap,
        # z_j at 32-39 read by a regular tensor_mul which honors offsets);
        # cols 320-447: the cos-combine matrix on partitions 0-7.
        wall = sb.tile([P, 448], bf16)
        nc.sync.dma_start(wall[:], w_ext[:])

        def wallj(j):
            return wall[:, 40 * j : 40 * j + 40]

        # ---- squares (bf16, split so each half starts on its ring's DMA)
        x2b = sb.tile([P, HALO], bf16)
        nc.vector.tensor_mul(x2b[0:64, :], xblk[0:64, :], xblk[0:64, :])
        nc.vector.tensor_mul(x2b[64:P, :], xblk[64:P, :], xblk[64:P, :])

        # n2 (rows 0-7) and zrow (rows 32-39) accumulate in one PSUM tile
        zn40 = psZ.tile([40, P], f32, tag="zn")
        for j in range(K):
            nc.tensor.matmul(
                zn40[:], wallj(j), x2b[:, j : j + P],
                start=(j == 0), stop=(j == K - 1),
            )

        # ---- ratio in block layout (no ACT, so no PWP table loads at all).
        # The custom-DVE reciprocal drops partition offsets on its input, so
        # n2 sits at rows 0-7; the regular tensor_mul handles the offset-32
        # zrow read correctly.
        inv = sb.tile([NBLK, P], f32)
        nc.vector.reciprocal_approx_fast(inv[:], zn40[0:NBLK, :])
        ratio = sb.tile([NBLK, P], bf16)
        nc.vector.tensor_mul(ratio[:], zn40[32:40, :], inv[:])

        # ---- out_blk[c*8+m, l'] = cos_c * ratio[m, l'] in ONE K=8 matmul;
        # the out-DMA's DRAM access pattern un-blocks the layout.
        outp = psO.tile([P, P], f32, tag="o")
        nc.tensor.matmul(
            outp[:], wall[0:NBLK, 320:448], ratio[:], start=True, stop=True
        )
        # evac + out-DMA split by halves so the first DMA issues earlier
        # and the two transfers pipeline
        outs = sb.tile([P, P], bf16)
        oap0 = _br.AP(out_ext, 0, [[LP, 8], [P, NBLK], [1, P]])
        oap1 = _br.AP(out_ext, 8 * LP, [[LP, 8], [P, NBLK], [1, P]])
        nc.vector.tensor_copy(outs[0:64, :], outp[0:64, :])
        nc.sync.dma_start(oap0, outs[0:64, :])
        nc.vector.tensor_copy(outs[64:P, :], outp[64:P, :])
        nc.scalar.dma_start(oap1, outs[64:P, :])

    nc.compile()
    return nc


def _structure(ent):
    """Return (s, ok): s = diag sign vector (patch-dim order) if the
    entangle matrix has the permutation structure that kills GX."""
    E128 = ent[:P, :]
    F, G = E128[:, :256], E128[:, 256:]
    GZ = F @ F.T - G @ G.T
    GX = F @ G.T + G @ F.T
    s = np.diag(GZ).copy()
    ok = (
        np.abs(GX).max() <= 1e-5
        and np.abs(GZ - np.diag(s)).max() <= 1e-5
        and np.abs(np.abs(s) - 1.0).max() <= 1e-5
    )
    return s, ok


def _fallback(x, theta, ent):
    """Exact dense numpy evaluation (never taken for the reference E)."""
    DIM = 512
    NQ = 9

    def ry(t):
        c, s = np.cos(t / 2.0), np.sin(t / 2.0)
        return np.array([[c, -s], [s, c]], dtype=np.float64)

    Rs = []
    for c in range(C_OUT):
        R = ry(theta[c, 0])
        for q in range(1, NQ):
            R = np.kron(R, ry(theta[c, q]))
        Rs.append(R)
    Rs = np.stack(Rs)
    Us = np.einsum("ij,cjk->cik", ent.astype(np.float64), Rs)
    idx = np.arange(L_OUT)[:, None] + np.arange(K)[None, :]
    patches = x[:, :, idx].transpose(0, 2, 1, 3).reshape(-1, C_IN * K)
    norms = np.maximum(np.linalg.norm(patches, axis=-1, keepdims=True), 1e-12)
    states = np.zeros((patches.shape[0], DIM))
    states[:, : C_IN * K] = patches / norms
    psi = np.einsum("cde,nd->cne", Us, states)
    bit0 = (np.arange(DIM) >> (NQ - 1)) & 1
    sign = np.where(bit0 == 0, 1.0, -1.0)
    out = np.einsum("cne,e->cn", psi * psi, sign)
    return (
        out.reshape(C_OUT, B, L_OUT).transpose(1, 0, 2).astype(np.float32)
    )


def _host_inputs(x, theta, s):
    import ml_dtypes

    xp = np.ones((B, C_IN, LXP), dtype=np.float32)
    xp[:, :, :L] = x
    # bf16 cols 40j+m' : ones (m==m'); cols 40j+32+m' : z_j (s[c*8+j]*(m==m'))
    # bf16 cols 320 + c*8+m~ on partition row m: cos(theta[c,0]) * (m==m~)
    wallC = np.zeros((P, 448), dtype=np.float32)
    m = np.arange(P) // 16
    c = np.arange(P) % 16
    for j in range(K):
        wallC[np.arange(P), 40 * j + m] = 1.0
        wallC[np.arange(P), 40 * j + 32 + m] = s[c * 8 + j]
    cos = np.cos(theta[:, 0])
    pc = np.arange(P) // NBLK
    pm = np.arange(P) % NBLK
    wallC[pm, 320 + np.arange(P)] = cos[pc]
    wallC = np.ascontiguousarray(wallC.astype(ml_dtypes.bfloat16))
    return xp, wallC


def _run(inputs, trace=False):
    from concourse.bass_utils import run_bass_kernel_spmd

    x = np.ascontiguousarray(np.asarray(inputs["x"], dtype=np.float32))
    theta = np.asarray(inputs["theta"], dtype=np.float32)
    ent = np.asarray(inputs["entangle_matrix"], dtype=np.float32)

    s, ok = _structure(ent)
    if not ok:  # pragma: no cover - safety net for non-reference entanglers
        return _fallback(x, theta, ent), None

    xp, wallC = _host_inputs(x, theta, s)

    if "nc" not in _CACHE:
        _CACHE["nc"] = _build_nc()
    nc = _CACHE["nc"]

    in_maps = [
        {"xp": np.ascontiguousarray(xp[b]), "wallC": wallC} for b in range(B)
    ]
    res = run_bass_kernel_spmd(
        nc, in_maps, core_ids=list(range(B)), trace=trace
    )
    out = np.stack([res.results[b]["out"][:, :L_OUT] for b in range(B)], axis=0)
    return np.ascontiguousarray(out.astype(np.float32)), res


def kernel(**inputs):
    out, _ = _run(inputs)
    return out


# revision 61
# speedup vs baseline: 1.1839x; 1.1653x over previous
"""DenseQConv1D Trainium2 kernel.

Math: the reference computes, per output channel c and patch p (128-dim im2col
column of x, normalized):
    out[c,p] = sum_e sign(e) * (s_p^T (E @ R_c)[:128,:])_e^2
with R_c = kron of 9 RY(theta[c,q]) rotations and sign(e) = Z on the MSB qubit.
Because every RY factor is orthogonal and the measurement only touches qubit 0,
with E128 = E[:128,:], F = E128[:,:256], G = E128[:,256:]:
    GZ = F F^T - G G^T,  GX = F G^T + G F^T   (both 128x128, theta-independent)
    out[c,p] = (cos t_c * p^T GZ p + sin t_c * p^T GX p) / ||p||^2,  t = theta[c,0]

For the ring-of-CNOTs entangle matrix E is a PERMUTATION matrix, under which
GZ collapses to diag(s) with s in {+-1} and GX == 0 identically (verified
against the jax reference to ~9e-7).  So, with patch dim d = c*8 + j,
    out[c~, l] = cos(theta[c~,0]) * zrow[l] / n2[l]
    zrow[l] = sum_j sum_c s[c*8+j] x[c, l+j]^2 ,  n2[l] = sum_j sum_c x[c,l+j]^2
The host computes s and cos(theta[:,0]) from the actual inputs (cheap numpy)
and falls back to an exact dense numpy evaluation if the structure ever fails
to hold.

Device kernel (per core, batch b): x is loaded ONCE (no 8x im2col DMA
duplication) in an 8-block layout xblk[(m,c), l'] = x[c, 128m + l'] with a
7-column halo; squares go to bf16 on DVE; the j-shifts of the im2col become
8 PSUM-accumulating bf16 matmuls with a block-diagonal stationary that
computes zrow (PSUM rows 0-7) and n2 (rows 32-39 - engines may only read
PSUM/SBUF at 32-aligned partition starts) in one pass:
    zn40 += wallC[:, 40j:40j+40]^T @ x2[:, j:j+128]
then a custom-DVE fast reciprocal, ratio = zrow*inv (bf16, block layout),
and a single K=8 matmul against a host-built cos*delta matrix that both
broadcasts cos over the 16 output channels and re-labels the blocks:
out_blk[c*8+m, l'] = cos_c * ratio[m, l']; the out-DMA's DRAM access
pattern un-blocks the layout.  The kernel has no ACT ops (no PWP table
loads); a dummy-matmul chain at kernel start warms the PE clock (HAM)
while the DMAs stream.  All host preprocessing (padding, the +-1/cos
stationaries) is O(KB) numpy glue; every per-element x computation runs
on device.

Sharding: batch dimension across the 8 cores (core b computes x[b]).
"""

import numpy as np

B = 8
C_IN = 16
C_OUT = 16
L = 1024
K = 8
L_OUT = L - K + 1  # 1017
LP = 1024  # padded patch count per core (cols 1017:1024 are dummy)
P = 128  # patch vector length = C_IN*K = partitions
LXP = 1040  # host-padded x row length (cols 1024: = 1.0)
NBLK = 8  # l-blocks of 128
HALO = 135  # 128 + K - 1
N_WARM = 22  # PE warmup matmuls

_CACHE = {}


def _build_nc():
    import bass_rust as _br
    import concourse.bacc as bacc
    import concourse.mybir as mybir
    import concourse.tile as tile

    f32 = mybir.dt.float32
    bf16 = mybir.dt.bfloat16

    nc = bacc.Bacc("TRN2", target_bir_lowering=False, debug=False)

    x_ext = nc.declare_dram_parameter("xp", [C_IN, LXP], f32, isOutput=False)
    w_ext = nc.declare_dram_parameter("wallC", [P, 308], bf16, isOutput=False)
    out_ext = nc.declare_dram_parameter("out", [C_OUT, LP], bf16, isOutput=True)

    with tile.TileContext(nc) as tc, tc.tile_pool(name="sb", bufs=1) as sb, \
            tc.tile_pool(name="psW", bufs=1, space="PSUM") as psW, \
            tc.tile_pool(name="psZ", bufs=1, space="PSUM") as psZ, \
            tc.tile_pool(name="psO", bufs=2, space="PSUM") as psO:
        # ---- PE warmup: release the HAM clock throttle while DMAs stream.
        warm = sb.tile([P, P], bf16)
        nc.vector.memset(warm[:], 0.0)
        wps = psW.tile([1, P], f32, tag="w")
        for _ in range(N_WARM):
            nc.tensor.matmul(
                wps[:], warm[:, 0:1], warm[:], start=True, stop=True
            )

        # ---- inputs: x once (8-block layout with halo) split across the two
        # HWDGE rings; wallC pipelined behind on ring A.
        # xblk[(m,c), l'] = xp[c, 128m + l'];  descriptors are 540B/partition
        xblk = sb.tile([P, HALO], f32)
        xap0 = _br.AP(x_ext, 0, [[128, 4], [LXP, C_IN], [1, HALO]])
        xap1 = _br.AP(x_ext, 128 * 4, [[128, 4], [LXP, C_IN], [1, HALO]])
        nc.sync.dma_start(xblk[0:64, :], xap0)
        nc.scalar.dma_start(xblk[64:P, :], xap1)

        # wallC: shift-j stationary = cols [20j, 20j+40): ones at +0..7 (n2
        # to PSUM partition 0 for the custom-DVE reciprocal) and z_j at
        # +32..39 (read by a regular tensor_mul which honors the offset-32).
        # PSUM rows 8-31 are never read, so the windows overlap with stride
        # 20 (n at 20j, z_j at 20j+32 are provably disjoint) - 180 cols
        # instead of 320.  Cols 180-307: the cos-combine matrix on
        # partitions 0-7.
        wall = sb.tile([P, 308], bf16)
        nc.sync.dma_start(wall[:], w_ext[:])

        def wallj(j):
            return wall[:, 20 * j : 20 * j + 40]

        # ---- squares (bf16, split so each half starts on its ring's DMA)
        x2b = sb.tile([P, HALO], bf16)
        nc.vector.tensor_mul(x2b[0:64, :], xblk[0:64, :], xblk[0:64, :])
        nc.vector.tensor_mul(x2b[64:P, :], xblk[64:P, :], xblk[64:P, :])

        # n2 (rows 0-7) and zrow (rows 32-39) accumulate in one PSUM tile
        zn40 = psZ.tile([40, P], f32, tag="zn")
        for j in range(K):
            nc.tensor.matmul(
                zn40[:], wallj(j), x2b[:, j : j + P],
                start=(j == 0), stop=(j == K - 1),
            )

        # ---- ratio in block layout (no ACT, so no PWP table loads at all).
        # The custom-DVE reciprocal drops partition offsets on its input, so
        # n2 sits at rows 0-7; the regular tensor_mul handles the offset-32
        # zrow read correctly.
        inv = sb.tile([NBLK, P], f32)
        nc.vector.reciprocal_approx_fast(inv[:], zn40[0:NBLK, :])
        ratio = sb.tile([NBLK, P], bf16)
        nc.vector.tensor_mul(ratio[:], zn40[32:40, :], inv[:])

        # ---- out_blk[c*8+m, l'] = cos_c * ratio[m, l'] in ONE K=8 matmul;
        # the out-DMA's DRAM access pattern un-blocks the layout.
        outp = psO.tile([P, P], f32, tag="o")
        nc.tensor.matmul(
            outp[:], wall[0:NBLK, 180:308], ratio[:], start=True, stop=True
        )
        # evac + out-DMA split by halves so the first DMA issues earlier
        # and the two transfers pipeline
        outs = sb.tile([P, P], bf16)
        oap0 = _br.AP(out_ext, 0, [[LP, 8], [P, NBLK], [1, P]])
        oap1 = _br.AP(out_ext, 8 * LP, [[LP, 8], [P, NBLK], [1, P]])
        nc.vector.tensor_copy(outs[0:64, :], outp[0:64, :])
        nc.sync.dma_start(oap0, outs[0:64, :])
        nc.vector.tensor_copy(outs[64:P, :], outp[64:P, :])
        nc.scalar.dma_start(oap1, outs[64:P, :])

    nc.compile()
    return nc


def _structure(ent):
    """Return (s, ok): s = diag sign vector (patch-dim order) if the
    entangle matrix has the permutation structure that kills GX."""
    E128 = ent[:P, :]
    F, G = E128[:, :256], E128[:, 256:]
    GZ = F @ F.T - G @ G.T
    GX = F @ G.T + G @ F.T
    s = np.diag(GZ).copy()
    ok = (
        np.abs(GX).max() <= 1e-5
        and np.abs(GZ - np.diag(s)).max() <= 1e-5
        and np.abs(np.abs(s) - 1.0).max() <= 1e-5
    )
    return s, ok


def _fallback(x, theta, ent):
    """Exact dense numpy evaluation (never taken for the reference E)."""
    DIM = 512
    NQ = 9

    def ry(t):
        c, s = np.cos(t / 2.0), np.sin(t / 2.0)
        return np.array([[c, -s], [s, c]], dtype=np.float64)

    Rs = []
    for c in range(C_OUT):
        R = ry(theta[c, 0])
        for q in range(1, NQ):
            R = np.kron(R, ry(theta[c, q]))
        Rs.append(R)
    Rs = np.stack(Rs)
    Us = np.einsum("ij,cjk->cik", ent.astype(np.float64), Rs)
    idx = np.arange(L_OUT)[:, None] + np.arange(K)[None, :]
    patches = x[:, :, idx].transpose(0, 2, 1, 3).reshape(-1, C_IN * K)
    norms = np.maximum(np.linalg.norm(patches, axis=-1, keepdims=True), 1e-12)
    states = np.zeros((patches.shape[0], DIM))
    states[:, : C_IN * K] = patches / norms
    psi = np.einsum("cde,nd->cne", Us, states)
    bit0 = (np.arange(DIM) >> (NQ - 1)) & 1
    sign = np.where(bit0 == 0, 1.0, -1.0)
    out = np.einsum("cne,e->cn", psi * psi, sign)
    return (
        out.reshape(C_OUT, B, L_OUT).transpose(1, 0, 2).astype(np.float32)
    )


def _host_inputs(x, theta, s):
    import ml_dtypes

    xp = np.ones((B, C_IN, LXP), dtype=np.float32)
    xp[:, :, :L] = x
    # overlapping stride-20 windows: ones at cols 20j+m', z_j at 20j+32+m'
    # bf16 cols 180 + c*8+m~ on partition row m: cos(theta[c,0]) * (m==m~)
    wallC = np.zeros((P, 308), dtype=np.float32)
    m = np.arange(P) // 16
    c = np.arange(P) % 16
    for j in range(K):
        wallC[np.arange(P), 20 * j + m] = 1.0
        wallC[np.arange(P), 20 * j + 32 + m] = s[c * 8 + j]
    cos = np.cos(theta[:, 0])
    pc = np.arange(P) // NBLK
    pm = np.arange(P) % NBLK
    wallC[pm, 180 + np.arange(P)] = cos[pc]
    wallC = np.ascontiguousarray(wallC.astype(ml_dtypes.bfloat16))
    return xp, wallC


def _run(inputs, trace=False):
    from concourse.bass_utils import run_bass_kernel_spmd

    x = np.ascontiguousarray(np.asarray(inputs["x"], dtype=np.float32))
    theta = np.asarray(inputs["theta"], dtype=np.float32)
    ent = np.asarray(inputs["entangle_matrix"], dtype=np.float32)

    s, ok = _structure(ent)
    if not ok:  # pragma: no cover - safety net for non-reference entanglers
        return _fallback(x, theta, ent), None

    xp, wallC = _host_inputs(x, theta, s)

    if "nc" not in _CACHE:
        _CACHE["nc"] = _build_nc()
    nc = _CACHE["nc"]

    in_maps = [
        {"xp": np.ascontiguousarray(xp[b]), "wallC": wallC} for b in range(B)
    ]
    res = run_bass_kernel_spmd(
        nc, in_maps, core_ids=list(range(B)), trace=trace
    )
    out = np.stack([res.results[b]["out"][:, :L_OUT] for b in range(B)], axis=0)
    return np.ascontiguousarray(out.astype(np.float32)), res


def kernel(**inputs):
    out, _ = _run(inputs)
    return out
